# revision 1
# baseline (speedup 1.0000x reference)
"""GATConv (PyG defaults, heads=1) Trainium2 Bass kernel.

Strategy (8 NeuronCores, node-parallel over destinations, no collectives):
  - Host: prepend self-loops, sort edges by destination, partition the
    destination space into blocks of 128 nodes. Within a block, slot each
    edge at (chunk k, partition dst%128): the per-chunk attention weight
    matrix is DIAGONAL, so per-edge destination indexing is free
    (partition index == dst-local index). Self-loops sort first in each
    run, so chunk 0 holds h[dst] rows, from which a_d is recomputed.
  - Each core owns ceil(NB/8) dst blocks and all edges into them; output
    row ranges are disjoint, so results are just concatenated.
  - Device, per core:
      Phase 1: h = x @ W (from host-transposed x^T), a_s = h @ att_src;
               write augmented node table rows [h(128) | a_s | pad]
               (192 f32 = 768 B) to DRAM.
      Phase 2: per block: dma_gather table rows for all edge slots;
               a_d = (chunk-0 rows) @ att_dst; z = a_s[src] + a_d;
               ex = exp(leaky_relu(z)); lhsT = diag(ex) via iota-compare;
               PSUM += diag(ex) @ h_rows over chunks; denom = row-sum of
               ex; out = psum * (1/denom) + bias.
  - Softmax is unshifted (no segment max): |logits| <~ 12 for this data
    distribution so exp() is far from fp32 overflow, and alpha is
    shift-invariant, matching the reference to fp32 noise.
  - Padded slots gather a sentinel row with a_s = -1e30 -> ex = 0 exactly,
    contributing nothing to numerator or denominator.
"""

import os
import sys

import numpy as np

sys.path.insert(0, "/opt/trn_rl_repo")

P = 128
ROWB = 192          # table row width in f32 (768 B; dma_gather needs %256B==0)
A_S_COL = P         # column holding a_s inside a table row
NEG_SLOPE = 0.2
NCORES = 8


def build_program(NPAD, NB, BPC, K, SEG, L16, dummy_part):
    import os as _os
    _STAGE = _os.environ.get("GAT_STAGE", "full")
    from concourse import bacc, bass, mybir, tile

    f32 = mybir.dt.float32
    i16 = mybir.dt.int16
    Alu = mybir.AluOpType
    Act = mybir.ActivationFunctionType
    NSEG = K // SEG

    nc = bacc.Bacc(None, num_swdge_queues=4)

    xT = nc.declare_dram_parameter("xT", [P, NPAD], f32, isOutput=False)
    Wp = nc.declare_dram_parameter("W", [P, P], f32, isOutput=False)
    asr = nc.declare_dram_parameter("att_src_rep", [P, P], f32, isOutput=False)
    adr = nc.declare_dram_parameter("att_dst_rep", [P, P], f32, isOutput=False)
    brp = nc.declare_dram_parameter("bias_rep", [P, P], f32, isOutput=False)
    idxp = nc.declare_dram_parameter("idxs", [P, L16], i16, isOutput=False)
    outp = nc.declare_dram_parameter("out", [BPC * P, P], f32, isOutput=True)
    table = nc.dram_tensor("table", [NPAD, ROWB], f32)

    with tile.TileContext(nc) as tc:
        with (
            tc.tile_pool(name="const", bufs=1) as cpool,
            tc.tile_pool(name="ps1", bufs=4, space="PSUM") as ps1,
            tc.tile_pool(name="junk", bufs=2) as jpool,
            tc.tile_pool(name="gseg", bufs=3) as gpool,
            tc.tile_pool(name="exz", bufs=2) as epool,
            tc.tile_pool(name="diag", bufs=4) as dpool,
            tc.tile_pool(name="ps2", bufs=2, space="PSUM") as ps2,
            tc.tile_pool(name="outb", bufs=2) as opool,
        ):
            # ---- constants / inputs resident in SBUF ----
            xT_sb = cpool.tile([P, NPAD], f32)
            nc.sync.dma_start(out=xT_sb[:], in_=xT[:])
            W_sb = cpool.tile([P, P], f32)
            nc.sync.dma_start(out=W_sb[:], in_=Wp[:])
            asr_sb = cpool.tile([P, P], f32)
            nc.sync.dma_start(out=asr_sb[:], in_=asr[:])
            adr_sb = cpool.tile([P, P], f32)
            nc.sync.dma_start(out=adr_sb[:], in_=adr[:])
            brp_sb = cpool.tile([P, P], f32)
            nc.sync.dma_start(out=brp_sb[:], in_=brp[:])
            idx_sb = cpool.tile([P, L16], i16)
            nc.sync.dma_start(out=idx_sb[:], in_=idxp[:])

            iota_row = cpool.tile([P, P], f32)
            nc.gpsimd.iota(iota_row[:], pattern=[[1, P]], base=0,
                           channel_multiplier=0,
                           allow_small_or_imprecise_dtypes=True)
            iota_col = cpool.tile([P, 1], f32)
            nc.gpsimd.iota(iota_col[:], pattern=[[1, 1]], base=0,
                           channel_multiplier=1,
                           allow_small_or_imprecise_dtypes=True)

            # ---- phase 1: h = x @ W, a_s; write node table (full rows) ----
            for nb in range(NB):
                ph = ps1.tile([P, P], f32, tag="ph")
                nc.tensor.matmul(out=ph[:], lhsT=xT_sb[:, nb * P:(nb + 1) * P],
                                 rhs=W_sb[:], start=True, stop=True)
                hsb = jpool.tile([P, ROWB], f32, tag="hsb")
                t0 = jpool.tile([P, P], f32, tag="t0")
                nc.vector.scalar_tensor_tensor(
                    out=t0[:], in0=ph[:], scalar=1.0, in1=asr_sb[:],
                    op0=Alu.mult, op1=Alu.mult,
                    accum_out=hsb[:, A_S_COL:A_S_COL + 1])
                nc.scalar.activation(out=hsb[:, 0:P], in_=ph[:], func=Act.Copy)
                nc.gpsimd.memset(hsb[:, A_S_COL + 1:ROWB], 0.0)
                if nb == NB - 1:
                    # dummy node: h-row is zero (xT zero-padded), so its
                    # accumulated a_s is 0; add -1e30 at its partition so
                    # padded slots' exp() underflows to exactly 0.
                    fix = jpool.tile([P, 1], f32, tag="fix")
                    nc.vector.tensor_scalar(
                        fix[:], iota_col[:, 0:1], float(dummy_part), -1e30,
                        Alu.is_equal, Alu.mult)
                    nc.vector.tensor_tensor(
                        out=hsb[:, A_S_COL:A_S_COL + 1],
                        in0=hsb[:, A_S_COL:A_S_COL + 1], in1=fix[:],
                        op=Alu.add)
                nc.sync.dma_start(out=table[nb * P:(nb + 1) * P, :],
                                  in_=hsb[:])

            # ---- phase 2: per-block gather + attention + aggregation ----
            for j in range(BPC if _STAGE != "phase1" else 0):
                po = ps2.tile([P, P], f32, tag="po")
                ex_blk = epool.tile([P, K, 1], f32, tag="ex")
                ad_col = epool.tile([P, 1], f32, tag="adc")
                for s in range(NSEG):
                    g = gpool.tile([P, SEG, ROWB], f32, tag="g")
                    c16 = (j * K + s * SEG) * P // 16
                    nc.gpsimd.dma_gather(
                        out_ap=g[:], in_ap=table[:],
                        idxs_ap=idx_sb[:, c16:c16 + SEG * P // 16],
                        num_idxs=SEG * P, num_idxs_reg=SEG * P,
                        elem_size=ROWB, single_packet=False,
                        queue_num=(j * NSEG + s) % 4)
                    if s == 0:
                        # chunk 0 is the self-loop chunk: rows are h[dst]
                        if _STAGE == "noad":
                            nc.vector.tensor_scalar(
                                ad_col[:], iota_col[:, 0:1], 0.0, None,
                                Alu.mult)
                        else:
                            tj = jpool.tile([P, P], f32, tag="t0")
                            nc.vector.scalar_tensor_tensor(
                                out=tj[:], in0=g[:, 0, 0:P], scalar=1.0,
                                in1=adr_sb[:], op0=Alu.mult, op1=Alu.mult,
                                accum_out=ad_col[:])
                    z = epool.tile([P, SEG, 1], f32, tag="z")
                    nc.vector.tensor_scalar(
                        z[:], g[:, :, A_S_COL:A_S_COL + 1],
                        ad_col[:, 0:1], None, Alu.add)
                    lz = epool.tile([P, SEG, 1], f32, tag="lz")
                    nc.vector.scalar_tensor_tensor(
                        out=lz[:], in0=z[:], scalar=NEG_SLOPE, in1=z[:],
                        op0=Alu.mult, op1=Alu.max)
                    nc.scalar.activation(
                        out=ex_blk[:, s * SEG:(s + 1) * SEG, :],
                        in_=lz[:], func=Act.Exp)
                    for k in range(SEG):
                        c = s * SEG + k
                        dg = dpool.tile([P, P], f32, tag="dg")
                        nc.vector.tensor_scalar(
                            dg[:], iota_row[:], iota_col[:, 0:1],
                            ex_blk[:, c:c + 1, 0:1], Alu.is_equal, Alu.mult)
                        nc.tensor.matmul(out=po[:], lhsT=dg[:],
                                         rhs=g[:, k, 0:P],
                                         start=(c == 0), stop=(c == K - 1))
                # normalize + bias
                dn = epool.tile([P, 1], f32, tag="dn")
                nc.vector.tensor_reduce(out=dn[:], in_=ex_blk[:],
                                        axis=mybir.AxisListType.XY,
                                        op=Alu.add)
                dn2 = epool.tile([P, 1], f32, tag="dn2")
                nc.vector.tensor_scalar(dn2[:], dn[:], 1e-30, None, Alu.max)
                rc = epool.tile([P, 1], f32, tag="rc")
                nc.vector.reciprocal(out=rc[:], in_=dn2[:])
                ob = opool.tile([P, P], f32, tag="ob")
                nc.vector.scalar_tensor_tensor(
                    out=ob[:], in0=po[:], scalar=rc[:, 0:1], in1=brp_sb[:],
                    op0=Alu.mult, op1=Alu.add)
                nc.sync.dma_start(out=outp[j * P:(j + 1) * P, :], in_=ob[:])

            if _STAGE == "phase1":
                zb = opool.tile([P, P], f32, tag="ob")
                nc.vector.tensor_scalar(zb[:], brp_sb[:], 1.0, None, Alu.mult)
                for j in range(BPC):
                    nc.sync.dma_start(out=outp[j * P:(j + 1) * P, :], in_=zb[:])

    nc.compile()
    return nc


def prepare(x, W, att_src, att_dst, bias, edge_index):
    """Host-side sharding/slotting. Returns (program args, per-core in_maps)."""
    x = np.asarray(x, dtype=np.float32)
    W = np.asarray(W, dtype=np.float32)
    att_src = np.asarray(att_src, dtype=np.float32)
    att_dst = np.asarray(att_dst, dtype=np.float32)
    bias = np.asarray(bias, dtype=np.float32)
    ei = np.asarray(edge_index)

    N, D = x.shape
    assert D == P

    # self-loops FIRST so they land at chunk 0 of every destination run
    loop = np.arange(N, dtype=np.int64)
    src = np.concatenate([loop, ei[0]]).astype(np.int32)
    dst = np.concatenate([loop, ei[1]]).astype(np.int32)
    order = np.argsort(dst, kind="stable")
    src_s, dst_s = src[order], dst[order]

    NB = (N + P - 1) // P
    if NB * P == N:        # need a spare row for the dummy/sentinel node
        NB += 1
    NPAD = NB * P
    BPC = (NB + NCORES - 1) // NCORES

    deg = np.bincount(dst_s, minlength=NPAD)
    Kraw = max(int(deg.max()), 1)
    NSEG = max(1, (Kraw + 25) // 26)   # cap SEG at 26 chunks per gather
    SEG = (Kraw + NSEG - 1) // NSEG
    K = NSEG * SEG

    DUMMY = N
    assert DUMMY < NPAD
    dummy_part = DUMMY - (NB - 1) * P

    grid = np.full((NB, K, P), DUMMY, dtype=np.int16)
    runstart = np.zeros(NPAD, dtype=np.int64)
    runstart[1:] = np.cumsum(deg)[:-1]
    k_e = np.arange(len(dst_s), dtype=np.int64) - runstart[dst_s]
    grid[dst_s // P, k_e, dst_s % P] = src_s

    L = BPC * K * P
    L16 = L // 16
    idx_inputs = []
    for c in range(NCORES):
        flat = np.full((BPC, K, P), DUMMY, dtype=np.int16)
        b0 = c * BPC
        nreal = max(0, min(BPC, NB - b0))
        if nreal > 0:
            flat[:nreal] = grid[b0:b0 + nreal]
        wrapped = flat.reshape(-1, 16).T.copy()
        # the 8 GPSIMD Q7 cores each read indices from their own group of
        # 16 partitions -> replicate the wrapped block into every group
        full = np.empty((P, L16), dtype=np.int16)
        for gp in range(P // 16):
            full[16 * gp:16 * (gp + 1)] = wrapped
        idx_inputs.append(full)

    xT = np.zeros((P, NPAD), dtype=np.float32)
    xT[:, :N] = x.T
    asr = np.broadcast_to(att_src, (P, P)).copy()
    adr = np.broadcast_to(att_dst, (P, P)).copy()
    brp = np.broadcast_to(bias, (P, P)).copy()

    in_maps = [{"xT": xT, "W": W, "att_src_rep": asr, "att_dst_rep": adr,
                "bias_rep": brp, "idxs": idx_inputs[c]} for c in range(NCORES)]
    return (NPAD, NB, BPC, K, SEG, L16, dummy_part), in_maps, (N, D)


def kernel(x, W, att_src, att_dst, bias, edge_index):
    from concourse.bass_utils import run_bass_kernel_spmd

    args, in_maps, (N, D) = prepare(x, W, att_src, att_dst, bias, edge_index)
    nc = build_program(*args)
    res = run_bass_kernel_spmd(nc, in_maps, list(range(NCORES)))

    BPC = args[2]
    out = np.empty((N, D), dtype=np.float32)
    for c in range(NCORES):
        rows0 = c * BPC * P
        rows1 = min(rows0 + BPC * P, N)
        if rows1 > rows0:
            out[rows0:rows1] = res.results[c]["out"][:rows1 - rows0]
    return out



# revision 2
# speedup vs baseline: 2.2135x; 2.2135x over previous
"""GATConv (PyG defaults, heads=1) Trainium2 Bass kernel.

Strategy (8 NeuronCores, node-parallel over destinations, no collectives):
  Key identity: out[dst] = (sum_e alpha_e * x[src_e]) @ W + bias -- the W
  GEMM distributes over the softmax-weighted aggregation, so we aggregate
  weighted *x* rows per destination first and multiply by W once per
  128-destination block. No per-edge h, no node-table gather.

  Host: prepend self-loops, sort edges by destination, partition the
  destination space into blocks of 128 nodes, slot each edge at
  (chunk k, partition dst%128) and pack x[src]^T... actually x[src] rows
  per chunk as fp16 tiles [slot=128, feat=128]. Self-loops land in chunk 0
  (giving h[dst]-equivalent rows for a_d). Padded slots carry x=0 plus an
  additive -1e30 logit mask.

  Device, per core (BPC blocks x K chunks):
    - a_s per chunk: stt(x_chunk * ws_rep) row-reduce  (ws = W @ att_src,
      computed on device once via a small matmul; likewise wd).
    - per block: z = a_s + a_d + mask; lrelu; per-dst max m (exact segment
      softmax like the reference); ex = exp(lz - m) in [0, 1] (fp16-safe).
    - dg_k = diag(ex_k) in fp16, built round-robin on Scalar(ACT)/GpSimd/
      Vector engines to balance load.
    - PE accumulates xaccT[c, d] += x_k[slot, c] * dg_k[slot, d] over all
      chunks in PSUM (one fp16 matmul per chunk).
    - block finalize: copy xaccT -> SBUF; po = xaccT.T @ W (fp32 matmul);
      out = po * (1/sum ex) + bias; DMA out.
  Each core owns BPC consecutive dst blocks; outputs are concatenated.
"""

import os
import sys

import numpy as np

sys.path.insert(0, "/opt/trn_rl_repo")

P = 128
NEG_SLOPE = 0.2
NCORES = 8

# dg-build engine assignment pattern (per chunk index mod len): balances the
# [128,128] scaled-onehot builds across Scalar(A), GpSimd(G), Vector(V).
DG_PATTERN = "AGAVAGAG"


def build_program(NB, BPC, K):
    from concourse import bacc, bass, mybir, tile

    f32 = mybir.dt.float32
    f16 = mybir.dt.float16
    Alu = mybir.AluOpType
    Act = mybir.ActivationFunctionType
    Ax = mybir.AxisListType

    nc = bacc.Bacc(None)

    xs = nc.declare_dram_parameter("xs", [P, BPC * K * P], f16, isOutput=False)
    msk = nc.declare_dram_parameter("msk", [P, BPC * K], f32, isOutput=False)
    Wp = nc.declare_dram_parameter("W", [P, P], f32, isOutput=False)
    WTp = nc.declare_dram_parameter("WT", [P, P], f32, isOutput=False)
    asr2 = nc.declare_dram_parameter("asr2", [P, P], f32, isOutput=False)
    adr2 = nc.declare_dram_parameter("adr2", [P, P], f32, isOutput=False)
    brp = nc.declare_dram_parameter("brp", [P, P], f32, isOutput=False)
    outp = nc.declare_dram_parameter("out", [BPC * P, P], f32, isOutput=True)

    with tile.TileContext(nc) as tc:
        with (
            tc.tile_pool(name="const", bufs=1) as cpool,
            tc.tile_pool(name="pssetup", bufs=2, space="PSUM") as pset,
            tc.tile_pool(name="xseg", bufs=3) as xpool,
            tc.tile_pool(name="blk", bufs=2) as bpool,
            tc.tile_pool(name="junk", bufs=2) as jpool,
            tc.tile_pool(name="dg", bufs=8) as dpool,
            tc.tile_pool(name="psacc", bufs=2, space="PSUM") as psa,
            tc.tile_pool(name="pso", bufs=2, space="PSUM") as pso,
            tc.tile_pool(name="fin", bufs=2) as fpool,
        ):
            # ---- constants ----
            W_sb = cpool.tile([P, P], f32)
            nc.sync.dma_start(out=W_sb[:], in_=Wp[:])
            WT_sb = cpool.tile([P, P], f32)
            nc.sync.dma_start(out=WT_sb[:], in_=WTp[:])
            asr_sb = cpool.tile([P, P], f32)
            nc.sync.dma_start(out=asr_sb[:], in_=asr2[:])
            adr_sb = cpool.tile([P, P], f32)
            nc.sync.dma_start(out=adr_sb[:], in_=adr2[:])
            brp_sb = cpool.tile([P, P], f32)
            nc.sync.dma_start(out=brp_sb[:], in_=brp[:])
            msk_sb = cpool.tile([P, BPC * K], f32)
            nc.sync.dma_start(out=msk_sb[:], in_=msk[:])

            iota_row = cpool.tile([P, P], f16)
            nc.gpsimd.iota(iota_row[:], pattern=[[1, P]], base=0,
                           channel_multiplier=0,
                           allow_small_or_imprecise_dtypes=True)
            iota_col = cpool.tile([P, 1], f32)
            nc.gpsimd.iota(iota_col[:], pattern=[[1, 1]], base=0,
                           channel_multiplier=1,
                           allow_small_or_imprecise_dtypes=True)
            # identity (0/1) in fp16 for ACT-built diag tiles
            I01 = cpool.tile([P, P], f16)
            nc.vector.tensor_scalar(I01[:], iota_row[:], iota_col[:, 0:1],
                                    None, Alu.is_equal)

            # ws_rep[p, c] = (W @ att_src)[c], wd_rep likewise (fp16 rows)
            ws_ps = pset.tile([P, P], f32, tag="wsps")
            nc.tensor.matmul(out=ws_ps[:], lhsT=asr_sb[:], rhs=WT_sb[:],
                             start=True, stop=True)
            ws_rep = cpool.tile([P, P], f16)
            nc.scalar.activation(out=ws_rep[:], in_=ws_ps[:], func=Act.Copy)
            wd_ps = pset.tile([P, P], f32, tag="wdps")
            nc.tensor.matmul(out=wd_ps[:], lhsT=adr_sb[:], rhs=WT_sb[:],
                             start=True, stop=True)
            wd_rep = cpool.tile([P, P], f16)
            nc.scalar.activation(out=wd_rep[:], in_=wd_ps[:], func=Act.Copy)

            # ---- main loop over destination blocks ----
            for j in range(BPC):
                xseg = xpool.tile([P, K * P], f16, tag="xseg")
                nc.sync.dma_start(out=xseg[:],
                                  in_=xs[:, j * K * P:(j + 1) * K * P])

                as_blk = bpool.tile([P, K], f32, tag="as")
                ad_col = bpool.tile([P, 1], f32, tag="ad")
                junk = jpool.tile([P, P], f16, tag="junk")
                for k in range(K):
                    nc.vector.scalar_tensor_tensor(
                        out=junk[:], in0=xseg[:, k * P:(k + 1) * P],
                        scalar=1.0, in1=ws_rep[:], op0=Alu.mult, op1=Alu.mult,
                        accum_out=as_blk[:, k:k + 1])
                    if k == 0:
                        jd = jpool.tile([P, P], f16, tag="junkd")
                        nc.vector.scalar_tensor_tensor(
                            out=jd[:], in0=xseg[:, 0:P], scalar=1.0,
                            in1=wd_rep[:], op0=Alu.mult, op1=Alu.mult,
                            accum_out=ad_col[:])

                # z = a_s + a_d + mask; lz = leaky_relu(z); m = rowmax;
                # ex = exp(lz - m)
                z_blk = bpool.tile([P, K], f32, tag="z")
                nc.vector.tensor_scalar(z_blk[:], as_blk[:], ad_col[:, 0:1],
                                        None, Alu.add)
                z2_blk = bpool.tile([P, K], f32, tag="z2")
                nc.vector.tensor_tensor(out=z2_blk[:], in0=z_blk[:],
                                        in1=msk_sb[:, j * K:(j + 1) * K],
                                        op=Alu.add)
                lz_blk = bpool.tile([P, K], f32, tag="lz")
                nc.vector.scalar_tensor_tensor(
                    out=lz_blk[:], in0=z2_blk[:], scalar=NEG_SLOPE,
                    in1=z2_blk[:], op0=Alu.mult, op1=Alu.max)
                m_col = bpool.tile([P, 1], f32, tag="m")
                nc.vector.tensor_reduce(out=m_col[:], in_=lz_blk[:],
                                        axis=Ax.X, op=Alu.max)
                lzm_blk = bpool.tile([P, K], f32, tag="lzm")
                nc.vector.tensor_scalar(lzm_blk[:], lz_blk[:], m_col[:, 0:1],
                                        None, Alu.subtract)
                ex_blk = bpool.tile([P, K], f32, tag="ex")
                nc.scalar.activation(out=ex_blk[:], in_=lzm_blk[:],
                                     func=Act.Exp)
                dn_col = bpool.tile([P, 1], f32, tag="dn")
                nc.vector.tensor_reduce(out=dn_col[:], in_=ex_blk[:],
                                        axis=Ax.X, op=Alu.add)
                rc_col = bpool.tile([P, 1], f32, tag="rc")
                nc.vector.reciprocal(out=rc_col[:], in_=dn_col[:])

                # per chunk: dg = diag(ex_k) fp16; xaccT += x_k^T-style matmul
                xacc_ps = psa.tile([P, P], f32, tag="xacc")
                for k in range(K):
                    dg = dpool.tile([P, P], f16, tag="dg")
                    eng = DG_PATTERN[k % len(DG_PATTERN)]
                    if eng == "A":
                        nc.scalar.activation(out=dg[:], in_=I01[:],
                                             func=Act.Copy,
                                             scale=ex_blk[:, k:k + 1])
                    elif eng == "G":
                        nc.gpsimd.tensor_scalar(
                            dg[:], iota_row[:], iota_col[:, 0:1],
                            ex_blk[:, k:k + 1], Alu.is_equal, Alu.mult)
                    else:
                        nc.vector.tensor_scalar(
                            dg[:], iota_row[:], iota_col[:, 0:1],
                            ex_blk[:, k:k + 1], Alu.is_equal, Alu.mult)
                    nc.tensor.matmul(out=xacc_ps[:],
                                     lhsT=xseg[:, k * P:(k + 1) * P],
                                     rhs=dg[:], start=(k == 0),
                                     stop=(k == K - 1))

                # finalize block: po = xaccT.T @ W; out = po * rc + bias
                xacc_sb = fpool.tile([P, P], f32, tag="xaccsb")
                nc.scalar.activation(out=xacc_sb[:], in_=xacc_ps[:],
                                     func=Act.Copy)
                po_ps = pso.tile([P, P], f32, tag="po")
                nc.tensor.matmul(out=po_ps[:], lhsT=xacc_sb[:], rhs=W_sb[:],
                                 start=True, stop=True)
                ob = fpool.tile([P, P], f32, tag="ob")
                nc.vector.scalar_tensor_tensor(
                    out=ob[:], in0=po_ps[:], scalar=rc_col[:, 0:1],
                    in1=brp_sb[:], op0=Alu.mult, op1=Alu.add)
                nc.sync.dma_start(out=outp[j * P:(j + 1) * P, :], in_=ob[:])

    nc.compile()
    return nc


def prepare(x, W, att_src, att_dst, bias, edge_index):
    """Host-side sharding/slotting. Returns (program args, per-core in_maps)."""
    x = np.asarray(x, dtype=np.float32)
    W = np.asarray(W, dtype=np.float32)
    att_src = np.asarray(att_src, dtype=np.float32)
    att_dst = np.asarray(att_dst, dtype=np.float32)
    bias = np.asarray(bias, dtype=np.float32)
    ei = np.asarray(edge_index)

    N, D = x.shape
    assert D == P

    # self-loops FIRST so they land at chunk 0 of every destination run
    loop = np.arange(N, dtype=np.int64)
    src = np.concatenate([loop, ei[0]]).astype(np.int32)
    dst = np.concatenate([loop, ei[1]]).astype(np.int32)
    order = np.argsort(dst, kind="stable")
    src_s, dst_s = src[order], dst[order]

    NB = (N + P - 1) // P
    BPC = (NB + NCORES - 1) // NCORES

    deg = np.bincount(dst_s, minlength=NB * P)
    K = max(int(deg.max()), 1)

    SENT = N  # sentinel source -> zero x row
    grid = np.full((NB, K, P), SENT, dtype=np.int32)
    runstart = np.zeros(NB * P, dtype=np.int64)
    runstart[1:] = np.cumsum(deg)[:-1]
    k_e = np.arange(len(dst_s), dtype=np.int64) - runstart[dst_s]
    grid[dst_s // P, k_e, dst_s % P] = src_s

    x16 = np.vstack([x.astype(np.float16), np.zeros((1, D), np.float16)])

    in_maps = []
    WT = np.ascontiguousarray(W.T)
    asr2c = np.ascontiguousarray(np.broadcast_to(att_src[:, None], (P, P)))
    adr2c = np.ascontiguousarray(np.broadcast_to(att_dst[:, None], (P, P)))
    brpc = np.ascontiguousarray(np.broadcast_to(bias, (P, P)))
    for c in range(NCORES):
        b0 = c * BPC
        nreal = max(0, min(BPC, NB - b0))
        g = np.full((BPC, K, P), SENT, dtype=np.int32)
        if nreal > 0:
            g[:nreal] = grid[b0:b0 + nreal]
        xsc = x16[g]  # [BPC, K, P(slot), D]
        xsc = np.ascontiguousarray(
            xsc.transpose(2, 0, 1, 3)).reshape(P, BPC * K * D)
        mc = np.where(g == SENT, np.float32(-1e30), np.float32(0.0))
        mc = np.ascontiguousarray(mc.transpose(2, 0, 1)).reshape(P, BPC * K)
        in_maps.append({"xs": xsc, "msk": mc, "W": W, "WT": WT,
                        "asr2": asr2c, "adr2": adr2c, "brp": brpc})
    return (NB, BPC, K), in_maps, (N, D)


def kernel(x, W, att_src, att_dst, bias, edge_index):
    from concourse.bass_utils import run_bass_kernel_spmd

    args, in_maps, (N, D) = prepare(x, W, att_src, att_dst, bias, edge_index)
    nc = build_program(*args)
    res = run_bass_kernel_spmd(nc, in_maps, list(range(NCORES)))

    BPC = args[1]
    out = np.empty((N, D), dtype=np.float32)
    for c in range(NCORES):
        rows0 = c * BPC * P
        rows1 = min(rows0 + BPC * P, N)
        if rows1 > rows0:
            out[rows0:rows1] = res.results[c]["out"][:rows1 - rows0]
    return out


# revision 6
# speedup vs baseline: 11.3260x; 5.1168x over previous
"""GATConv (PyG defaults, heads=1) Trainium2 Bass kernel.

Strategy (8 NeuronCores, edge-parallel over dst-sorted slots):
  Identity: out[dst] = (sum_e alpha_e * x[src_e]) @ W + bias -- the W GEMM
  distributes over the softmax-weighted aggregation, so we aggregate
  weighted *x* rows per destination first and multiply by W once per
  128-destination block.

  Host: prepend self-loops, sort edges by destination. Destinations are
  ordered by degree (descending) and packed into 128-node blocks so each
  block's chunk count K_b ~= its mean degree (kills max-degree padding).
  Blocks are dealt snake-wise to the 8 cores; the per-position chunk
  count KS[p] (max over cores) is compile-time so one SPMD program fits
  all cores. Each edge occupies (chunk k, partition q=dst slot); chunk 0
  is the self-loop. Host packs TWO fp16 layouts of x[src] per chunk:
    xs  [slot=128, feat=128]  (lhsT of the aggregation matmul)
    xsT [feat=128, slot=128]  (lhsT of the per-slot logit matmul)
  Padded slots carry x=0 plus an additive -1e30 logit mask.

  Device, per core (BPC blocks, K_j chunks each):
    - a_s per slot via PE: matmul(lhsT=xsT_k, rhs=W@att_src) -> [slot,1]
      PSUM column (W@att_src / W@att_dst computed on device once).
    - per block: z = a_s + a_d + mask; leaky-relu; per-dst max m (exact
      segment softmax, like the reference); ex = exp(lz - m) in [0,1].
    - dg_k = diag(ex_k) fp16, built on Vector/Scalar engines (tunable
      split); PE accumulates xaccT[c,d] += xs_k[slot,c]*dg_k[slot,d].
    - finalize: po = xaccT.T @ W (fp32); out = po/sum(ex) + bias.
  Outputs are un-permuted on the host (degree-sort is a permutation).
"""

import os
import sys

import numpy as np

sys.path.insert(0, "/opt/trn_rl_repo")

P = 128
NEG_SLOPE = 0.2
NCORES = 8

# dg-build engine split: V=Vector(DVE), A=Scalar(ACT), cycled per chunk
DG_PATTERN = "VVAVVAVA"


def build_program(BPC, KS, CT):
    from concourse import bacc, bass, mybir, tile

    f32 = mybir.dt.float32
    f16 = mybir.dt.float16
    Alu = mybir.AluOpType
    Act = mybir.ActivationFunctionType
    Ax = mybir.AxisListType
    KMAX = max(KS)
    offs = np.concatenate([[0], np.cumsum(KS)]).astype(int)

    nc = bacc.Bacc(None)

    xs = nc.declare_dram_parameter("xs", [P, CT * P], f16, isOutput=False)
    xsT = nc.declare_dram_parameter("xsT", [P, CT * P], f16, isOutput=False)
    msk = nc.declare_dram_parameter("msk", [P, CT], f32, isOutput=False)
    Wp = nc.declare_dram_parameter("W", [P, P], f32, isOutput=False)
    WTp = nc.declare_dram_parameter("WT", [P, P], f32, isOutput=False)
    ascp = nc.declare_dram_parameter("asc", [P, 2], f32, isOutput=False)
    brp = nc.declare_dram_parameter("brp", [P, P], f32, isOutput=False)
    outp = nc.declare_dram_parameter("out", [BPC * P, P], f32, isOutput=True)

    with tile.TileContext(nc) as tc:
        with (
            tc.tile_pool(name="const", bufs=1) as cpool,
            tc.tile_pool(name="pset", bufs=1, space="PSUM") as pset,
            tc.tile_pool(name="xseg", bufs=2) as xpool,
            tc.tile_pool(name="xTseg", bufs=2) as xTpool,
            tc.tile_pool(name="psas", bufs=2, space="PSUM") as psas,
            tc.tile_pool(name="blk", bufs=2) as bpool,
            tc.tile_pool(name="dg", bufs=8) as dpool,
            tc.tile_pool(name="psacc", bufs=2, space="PSUM") as psa,
            tc.tile_pool(name="pso", bufs=2, space="PSUM") as pso,
            tc.tile_pool(name="fin", bufs=2) as fpool,
        ):
            # ---- constants ----
            W_sb = cpool.tile([P, P], f32)
            nc.sync.dma_start(out=W_sb[:], in_=Wp[:])
            WT_sb = cpool.tile([P, P], f32)
            nc.sync.dma_start(out=WT_sb[:], in_=WTp[:])
            asc_sb = cpool.tile([P, 2], f32)
            nc.sync.dma_start(out=asc_sb[:], in_=ascp[:])
            brp_sb = cpool.tile([P, P], f32)
            nc.sync.dma_start(out=brp_sb[:], in_=brp[:])
            msk_sb = cpool.tile([P, CT], f32)
            nc.sync.dma_start(out=msk_sb[:], in_=msk[:])

            iota_row = cpool.tile([P, P], f16)
            nc.gpsimd.iota(iota_row[:], pattern=[[1, P]], base=0,
                           channel_multiplier=0,
                           allow_small_or_imprecise_dtypes=True)
            iota_col = cpool.tile([P, 1], f32)
            nc.gpsimd.iota(iota_col[:], pattern=[[1, 1]], base=0,
                           channel_multiplier=1,
                           allow_small_or_imprecise_dtypes=True)
            I01 = cpool.tile([P, P], f16)
            nc.vector.tensor_scalar(I01[:], iota_row[:], iota_col[:, 0:1],
                                    None, Alu.is_equal)

            # wsd[c, 0:2] = [W @ att_src, W @ att_dst][c]
            wsd_ps = pset.tile([P, 2], f32, tag="wsd")
            nc.tensor.matmul(out=wsd_ps[:], lhsT=WT_sb[:], rhs=asc_sb[:],
                             start=True, stop=True)
            wsd16 = cpool.tile([P, 2], f16)
            nc.scalar.activation(out=wsd16[:], in_=wsd_ps[:], func=Act.Copy)

            # ---- per-block state emitted in a 1-block software pipeline:
            # as-phase(j) runs on PE before agg-phase(j-1) so the z->ex->dg
            # chain of block j overlaps the aggregation matmuls of j-1.
            state = [None] * BPC

            def as_phase(j):
                K = KS[j]
                o = offs[j]
                xseg = xpool.tile([P, KMAX * P], f16, tag="xseg")
                nc.sync.dma_start(out=xseg[:, 0:K * P],
                                  in_=xs[:, o * P:(o + K) * P])
                xTseg = xTpool.tile([P, KMAX * P], f16, tag="xTseg")
                nc.sync.dma_start(out=xTseg[:, 0:K * P],
                                  in_=xsT[:, o * P:(o + K) * P])
                as_ps = psas.tile([P, KMAX + 1], f32, tag="asps")
                for k in range(K):
                    nc.tensor.matmul(out=as_ps[:, k:k + 1],
                                     lhsT=xTseg[:, k * P:(k + 1) * P],
                                     rhs=wsd16[:, 0:1], start=True, stop=True,
                                     skip_group_check=True)
                ad_ps = as_ps[:, KMAX:KMAX + 1]
                nc.tensor.matmul(out=ad_ps, lhsT=xTseg[:, 0:P],
                                 rhs=wsd16[:, 1:2], start=True, stop=True,
                                 skip_group_check=True)

                # z = a_s + a_d + mask; lz = lrelu(z); ex = exp(lz - max)
                ad_sb = bpool.tile([P, 1], f32, tag="adsb")
                nc.vector.tensor_scalar(ad_sb[:], ad_ps, 0.0, None,
                                        Alu.add)
                z_blk = bpool.tile([P, KMAX], f32, tag="z")
                nc.vector.tensor_scalar(z_blk[:, 0:K], as_ps[:, 0:K],
                                        ad_sb[:, 0:1], None, Alu.add)
                z2_blk = bpool.tile([P, KMAX], f32, tag="z2")
                nc.vector.tensor_tensor(out=z2_blk[:, 0:K], in0=z_blk[:, 0:K],
                                        in1=msk_sb[:, o:o + K], op=Alu.add)
                lz_blk = bpool.tile([P, KMAX], f32, tag="lz")
                nc.vector.scalar_tensor_tensor(
                    out=lz_blk[:, 0:K], in0=z2_blk[:, 0:K], scalar=NEG_SLOPE,
                    in1=z2_blk[:, 0:K], op0=Alu.mult, op1=Alu.max)
                m_col = bpool.tile([P, 1], f32, tag="m")
                nc.vector.tensor_reduce(out=m_col[:], in_=lz_blk[:, 0:K],
                                        axis=Ax.X, op=Alu.max)
                lzm_blk = bpool.tile([P, KMAX], f32, tag="lzm")
                nc.vector.tensor_scalar(lzm_blk[:, 0:K], lz_blk[:, 0:K],
                                        m_col[:, 0:1], None, Alu.subtract)
                ex_blk = bpool.tile([P, KMAX], f32, tag="ex")
                nc.scalar.activation(out=ex_blk[:, 0:K], in_=lzm_blk[:, 0:K],
                                     func=Act.Exp)
                dn_col = bpool.tile([P, 1], f32, tag="dn")
                nc.vector.tensor_reduce(out=dn_col[:], in_=ex_blk[:, 0:K],
                                        axis=Ax.X, op=Alu.add)
                rc_col = bpool.tile([P, 1], f32, tag="rc")
                nc.vector.reciprocal(out=rc_col[:], in_=dn_col[:])
                state[j] = (xseg, ex_blk, rc_col)

            def agg_phase(j):
                K = KS[j]
                xseg, ex_blk, rc_col = state[j]
                state[j] = None
                xacc_ps = psa.tile([P, P], f32, tag="xacc")
                for k in range(K):
                    dg = dpool.tile([P, P], f16, tag="dg")
                    if DG_PATTERN[k % len(DG_PATTERN)] == "A":
                        nc.scalar.activation(out=dg[:], in_=I01[:],
                                             func=Act.Copy,
                                             scale=ex_blk[:, k:k + 1])
                    else:
                        nc.vector.tensor_scalar(
                            dg[:], iota_row[:], iota_col[:, 0:1],
                            ex_blk[:, k:k + 1], Alu.is_equal, Alu.mult)
                    nc.tensor.matmul(out=xacc_ps[:],
                                     lhsT=xseg[:, k * P:(k + 1) * P],
                                     rhs=dg[:], start=(k == 0),
                                     stop=(k == K - 1),
                                     skip_group_check=True)

                xacc_sb = fpool.tile([P, P], f32, tag="xaccsb")
                nc.scalar.activation(out=xacc_sb[:], in_=xacc_ps[:],
                                     func=Act.Copy)
                po_ps = pso.tile([P, P], f32, tag="po")
                nc.tensor.matmul(out=po_ps[:], lhsT=xacc_sb[:], rhs=W_sb[:],
                                 start=True, stop=True, skip_group_check=True)
                ob = fpool.tile([P, P], f32, tag="ob")
                nc.vector.scalar_tensor_tensor(
                    out=ob[:], in0=po_ps[:], scalar=rc_col[:, 0:1],
                    in1=brp_sb[:], op0=Alu.mult, op1=Alu.add)
                nc.sync.dma_start(out=outp[j * P:(j + 1) * P, :], in_=ob[:])

            for j in range(BPC):
                as_phase(j)
                if j > 0:
                    agg_phase(j - 1)
            agg_phase(BPC - 1)

    nc.compile()
    return nc


def prepare(x, W, att_src, att_dst, bias, edge_index):
    """Host-side sharding/slotting. Returns (program args, per-core in_maps,
    (N, D, node_map)) where node_map[c][row] = destination node of that
    output row (or -1 for padding)."""
    x = np.asarray(x, dtype=np.float32)
    W = np.asarray(W, dtype=np.float32)
    att_src = np.asarray(att_src, dtype=np.float32)
    att_dst = np.asarray(att_dst, dtype=np.float32)
    bias = np.asarray(bias, dtype=np.float32)
    ei = np.asarray(edge_index)

    N, D = x.shape
    assert D == P

    # self-loops FIRST so they land at chunk 0 of every destination run
    loop = np.arange(N, dtype=np.int64)
    src = np.concatenate([loop, ei[0]]).astype(np.int32)
    dst = np.concatenate([loop, ei[1]]).astype(np.int32)
    order = np.argsort(dst, kind="stable")
    src_s, dst_s = src[order], dst[order]

    deg = np.bincount(dst_s, minlength=N)
    runstart = np.zeros(N, dtype=np.int64)
    runstart[1:] = np.cumsum(deg)[:-1]

    NB = (N + P - 1) // P
    BPC = (NB + NCORES - 1) // NCORES
    NBP = BPC * NCORES

    # order destinations by degree (desc) -> blocks of near-equal degree
    perm = np.argsort(-deg, kind="stable").astype(np.int32)
    SENT = N
    blk_nodes = np.full((NBP, P), SENT, dtype=np.int32)
    blk_nodes.reshape(-1)[:N] = perm
    blk_deg = np.where(blk_nodes == SENT, 0,
                       deg[np.minimum(blk_nodes, N - 1)])
    blk_K = blk_deg.max(axis=1)

    # snake-deal blocks (sorted by K desc == index order) to (pos, core)
    KS = []
    core_blocks = [[] for _ in range(NCORES)]
    for p in range(BPC):
        idxs = list(range(p * NCORES, (p + 1) * NCORES))
        if p % 2 == 1:
            idxs = idxs[::-1]
        for c in range(NCORES):
            core_blocks[c].append(idxs[c])
        KS.append(int(max(max(blk_K[i] for i in idxs), 1)))
    CT = int(sum(KS))
    offs = np.concatenate([[0], np.cumsum(KS)]).astype(int)

    x16 = np.vstack([x.astype(np.float16), np.zeros((1, D), np.float16)])

    WT = np.ascontiguousarray(W.T)
    asc = np.ascontiguousarray(np.stack([att_src, att_dst], axis=1))
    brpc = np.ascontiguousarray(np.broadcast_to(bias, (P, P)))

    in_maps = []
    node_map = []
    for c in range(NCORES):
        grid = np.full((CT, P), SENT, dtype=np.int32)
        rows = np.empty(BPC * P, dtype=np.int32)
        for p in range(BPC):
            b = core_blocks[c][p]
            nodes = blk_nodes[b]
            rows[p * P:(p + 1) * P] = np.where(nodes == SENT, -1, nodes)
            real = np.nonzero(nodes != SENT)[0]
            if len(real) == 0:
                continue
            nd = nodes[real]
            dg_ = deg[nd]
            # edge t of node nd -> chunk offs[p]+t, partition real slot
            tot = dg_.sum()
            qs = np.repeat(real, dg_)
            ks = (np.arange(tot) -
                  np.repeat(np.cumsum(dg_) - dg_, dg_)).astype(np.int64)
            es = (np.repeat(runstart[nd], dg_) + ks)
            grid[offs[p] + ks, qs] = src_s[es]
        xg = x16[grid]  # [CT, P(slot), D]
        xsc = np.ascontiguousarray(xg.transpose(1, 0, 2)).reshape(P, CT * P)
        xTc = np.ascontiguousarray(xg.transpose(2, 0, 1)).reshape(P, CT * P)
        mc = np.where(grid == SENT, np.float32(-1e30), np.float32(0.0))
        mc = np.ascontiguousarray(mc.T)
        in_maps.append({"xs": xsc, "xsT": xTc, "msk": mc, "W": W, "WT": WT,
                        "asc": asc, "brp": brpc})
        node_map.append(rows)
    return (BPC, KS, CT), in_maps, (N, D, node_map)


def kernel(x, W, att_src, att_dst, bias, edge_index):
    from concourse.bass_utils import run_bass_kernel_spmd

    args, in_maps, (N, D, node_map) = prepare(x, W, att_src, att_dst, bias,
                                              edge_index)
    nc = build_program(*args)
    res = run_bass_kernel_spmd(nc, in_maps, list(range(NCORES)))

    out = np.empty((N, D), dtype=np.float32)
    for c in range(NCORES):
        rows = node_map[c]
        valid = rows >= 0
        out[rows[valid]] = res.results[c]["out"][valid]
    return out


# revision 9
# speedup vs baseline: 13.0509x; 1.1523x over previous
"""GATConv (PyG defaults, heads=1) Trainium2 Bass kernel.

Strategy (8 NeuronCores, edge-parallel over dst-sorted slots):
  Identity: out[dst] = (sum_e alpha_e * x[src_e]) @ W + bias -- the W GEMM
  distributes over the softmax-weighted aggregation, so we aggregate
  weighted *x* rows per destination first and multiply by W once per
  128-destination block.

  Host: prepend self-loops, sort edges by destination. Destinations are
  ordered by degree (descending) and packed into 128-node blocks so each
  block's chunk count K_b ~= its mean degree (kills max-degree padding).
  Blocks are dealt snake-wise to the 8 cores; the per-position chunk
  count KS[p] (max over cores) is compile-time so one SPMD program fits
  all cores. Each edge occupies (chunk k, partition q=dst slot); chunk 0
  is the self-loop. Host packs TWO fp16 layouts of x[src] per chunk:
    xs  [slot=128, feat=128]  (lhsT of the aggregation matmul)
    xsT [feat=128, slot=128]  (lhsT of the per-slot logit matmul)
  Padded slots carry x=0 plus an additive -1e30 logit mask.

  Device, per core (BPC blocks, K_j chunks each):
    - a_s per slot via PE: matmul(lhsT=xsT_k, rhs=W@att_src) -> [slot,1]
      PSUM column (W@att_src / W@att_dst computed on device once).
    - per block: z = a_s + a_d + mask; leaky-relu; per-dst max m (exact
      segment softmax, like the reference); ex = exp(lz - m) in [0,1].
    - dg_k = diag(ex_k) fp16, built on Vector/Scalar engines (tunable
      split); PE accumulates xaccT[c,d] += xs_k[slot,c]*dg_k[slot,d].
    - finalize: po = xaccT.T @ W (fp32); out = po/sum(ex) + bias.
  Outputs are un-permuted on the host (degree-sort is a permutation).
"""

import os
import sys

import numpy as np

sys.path.insert(0, "/opt/trn_rl_repo")

P = 128
NEG_SLOPE = 0.2
NCORES = 8

# dg-build engine split: V=Vector(DVE), A=Scalar(ACT), cycled per chunk
DG_PATTERN = "VVAVVAVA"


def build_program(BPC, KS, CT):
    from concourse import bacc, bass, mybir, tile

    f32 = mybir.dt.float32
    f16 = mybir.dt.float16
    Alu = mybir.AluOpType
    Act = mybir.ActivationFunctionType
    Ax = mybir.AxisListType
    KMAX = max(KS)
    offs = np.concatenate([[0], np.cumsum(KS)]).astype(int)

    nc = bacc.Bacc(None)

    xs = nc.declare_dram_parameter("xs", [P, CT * P], f16, isOutput=False)
    xsT = nc.declare_dram_parameter("xsT", [P, CT * P], f16, isOutput=False)
    msk = nc.declare_dram_parameter("msk", [P, CT], f32, isOutput=False)
    Wp = nc.declare_dram_parameter("W", [P, P], f32, isOutput=False)
    WTp = nc.declare_dram_parameter("WT", [P, P], f32, isOutput=False)
    ascp = nc.declare_dram_parameter("asc", [P, 2], f32, isOutput=False)
    brp = nc.declare_dram_parameter("brp", [P, P], f32, isOutput=False)
    outp = nc.declare_dram_parameter("out", [BPC * P, P], f32, isOutput=True)

    with tile.TileContext(nc) as tc:
        with (
            tc.tile_pool(name="const", bufs=1) as cpool,
            tc.tile_pool(name="pset", bufs=1, space="PSUM") as pset,
            tc.tile_pool(name="xseg", bufs=2) as xpool,
            tc.tile_pool(name="xTseg", bufs=2) as xTpool,
            tc.tile_pool(name="psas", bufs=2, space="PSUM") as psas,
            tc.tile_pool(name="blk", bufs=2) as bpool,
            tc.tile_pool(name="dg", bufs=8) as dpool,
            tc.tile_pool(name="psacc", bufs=2, space="PSUM") as psa,
            tc.tile_pool(name="pso", bufs=2, space="PSUM") as pso,
            tc.tile_pool(name="fin", bufs=2) as fpool,
        ):
            # ---- constants ----
            W_sb = cpool.tile([P, P], f32)
            nc.sync.dma_start(out=W_sb[:], in_=Wp[:])
            WT_sb = cpool.tile([P, P], f32)
            nc.sync.dma_start(out=WT_sb[:], in_=WTp[:])
            asc_sb = cpool.tile([P, 2], f32)
            nc.sync.dma_start(out=asc_sb[:], in_=ascp[:])
            brp_sb = cpool.tile([P, P], f32)
            nc.sync.dma_start(out=brp_sb[:], in_=brp[:])
            msk_sb = cpool.tile([P, CT], f32)
            nc.sync.dma_start(out=msk_sb[:], in_=msk[:])

            iota_row = cpool.tile([P, P], f16)
            nc.gpsimd.iota(iota_row[:], pattern=[[1, P]], base=0,
                           channel_multiplier=0,
                           allow_small_or_imprecise_dtypes=True)
            iota_col = cpool.tile([P, 1], f32)
            nc.gpsimd.iota(iota_col[:], pattern=[[1, 1]], base=0,
                           channel_multiplier=1,
                           allow_small_or_imprecise_dtypes=True)
            I01 = cpool.tile([P, P], f16)
            nc.vector.tensor_scalar(I01[:], iota_row[:], iota_col[:, 0:1],
                                    None, Alu.is_equal)

            # wsd[c, 0:2] = [W @ att_src, W @ att_dst][c]
            wsd_ps = pset.tile([P, 2], f32, tag="wsd")
            nc.tensor.matmul(out=wsd_ps[:], lhsT=WT_sb[:], rhs=asc_sb[:],
                             start=True, stop=True)
            wsd16 = cpool.tile([P, 2], f16)
            nc.scalar.activation(out=wsd16[:], in_=wsd_ps[:], func=Act.Copy)

            # ---- software pipeline over blocks: per iteration j emit
            #   A_j  (DMA + per-slot logit matmuls on PE)
            #   G_j-1 (dg builds + aggregation matmuls + finalize)
            #   Z_j  (z -> lrelu -> max -> exp -> denom batch)
            # so the DVE/ACT dg work of block j-1 is queued BEFORE the Z_j
            # ops that stall on A_j's matmuls (avoids head-of-line blocking).
            state = [None] * BPC

            def as_phase(j):
                K = KS[j]
                o = offs[j]
                xseg = xpool.tile([P, KMAX * P], f16, tag="xseg")
                nc.sync.dma_start(out=xseg[:, 0:K * P],
                                  in_=xs[:, o * P:(o + K) * P])
                xTseg = xTpool.tile([P, KMAX * P], f16, tag="xTseg")
                nc.sync.dma_start(out=xTseg[:, 0:K * P],
                                  in_=xsT[:, o * P:(o + K) * P])
                as_ps = psas.tile([P, KMAX + 1], f32, tag="asps")
                for k in range(K):
                    nc.tensor.matmul(out=as_ps[:, k:k + 1],
                                     lhsT=xTseg[:, k * P:(k + 1) * P],
                                     rhs=wsd16[:, 0:1], start=True, stop=True,
                                     skip_group_check=True)
                ad_ps = as_ps[:, KMAX:KMAX + 1]
                nc.tensor.matmul(out=ad_ps, lhsT=xTseg[:, 0:P],
                                 rhs=wsd16[:, 1:2], start=True, stop=True,
                                 skip_group_check=True)
                state[j] = [xseg, as_ps, None, None]

            def z_phase(j):
                K = KS[j]
                o = offs[j]
                as_ps = state[j][1]
                ad_ps = as_ps[:, KMAX:KMAX + 1]
                # z = a_s + a_d + mask; lz = lrelu(z); ex = exp(lz - max)
                ad_sb = bpool.tile([P, 1], f32, tag="adsb")
                nc.vector.tensor_scalar(ad_sb[:], ad_ps, 0.0, None,
                                        Alu.add)
                z_blk = bpool.tile([P, KMAX], f32, tag="z")
                nc.vector.tensor_scalar(z_blk[:, 0:K], as_ps[:, 0:K],
                                        ad_sb[:, 0:1], None, Alu.add)
                z2_blk = bpool.tile([P, KMAX], f32, tag="z2")
                nc.vector.tensor_tensor(out=z2_blk[:, 0:K], in0=z_blk[:, 0:K],
                                        in1=msk_sb[:, o:o + K], op=Alu.add)
                lz_blk = bpool.tile([P, KMAX], f32, tag="lz")
                nc.vector.scalar_tensor_tensor(
                    out=lz_blk[:, 0:K], in0=z2_blk[:, 0:K], scalar=NEG_SLOPE,
                    in1=z2_blk[:, 0:K], op0=Alu.mult, op1=Alu.max)
                m_col = bpool.tile([P, 1], f32, tag="m")
                nc.vector.tensor_reduce(out=m_col[:], in_=lz_blk[:, 0:K],
                                        axis=Ax.X, op=Alu.max)
                lzm_blk = bpool.tile([P, KMAX], f32, tag="lzm")
                nc.vector.tensor_scalar(lzm_blk[:, 0:K], lz_blk[:, 0:K],
                                        m_col[:, 0:1], None, Alu.subtract)
                ex_blk = bpool.tile([P, KMAX], f32, tag="ex")
                nc.scalar.activation(out=ex_blk[:, 0:K], in_=lzm_blk[:, 0:K],
                                     func=Act.Exp)
                dn_col = bpool.tile([P, 1], f32, tag="dn")
                nc.vector.tensor_reduce(out=dn_col[:], in_=ex_blk[:, 0:K],
                                        axis=Ax.X, op=Alu.add)
                rc_col = bpool.tile([P, 1], f32, tag="rc")
                nc.vector.reciprocal(out=rc_col[:], in_=dn_col[:])
                state[j][2] = ex_blk
                state[j][3] = rc_col

            def agg_phase(j):
                K = KS[j]
                xseg, _, ex_blk, rc_col = state[j]
                state[j] = None
                xacc_ps = psa.tile([P, P], f32, tag="xacc")
                for k in range(K):
                    dg = dpool.tile([P, P], f16, tag="dg")
                    if DG_PATTERN[k % len(DG_PATTERN)] == "A":
                        nc.scalar.activation(out=dg[:], in_=I01[:],
                                             func=Act.Copy,
                                             scale=ex_blk[:, k:k + 1])
                    else:
                        nc.vector.tensor_scalar(
                            dg[:], iota_row[:], iota_col[:, 0:1],
                            ex_blk[:, k:k + 1], Alu.is_equal, Alu.mult)
                    nc.tensor.matmul(out=xacc_ps[:],
                                     lhsT=xseg[:, k * P:(k + 1) * P],
                                     rhs=dg[:], start=(k == 0),
                                     stop=(k == K - 1),
                                     skip_group_check=True)

                xacc_sb = fpool.tile([P, P], f32, tag="xaccsb")
                nc.scalar.activation(out=xacc_sb[:], in_=xacc_ps[:],
                                     func=Act.Copy)
                po_ps = pso.tile([P, P], f32, tag="po")
                nc.tensor.matmul(out=po_ps[:], lhsT=xacc_sb[:], rhs=W_sb[:],
                                 start=True, stop=True, skip_group_check=True)
                ob = fpool.tile([P, P], f32, tag="ob")
                nc.vector.scalar_tensor_tensor(
                    out=ob[:], in0=po_ps[:], scalar=rc_col[:, 0:1],
                    in1=brp_sb[:], op0=Alu.mult, op1=Alu.add)
                nc.sync.dma_start(out=outp[j * P:(j + 1) * P, :], in_=ob[:])

            for j in range(BPC):
                as_phase(j)
                if j > 0:
                    agg_phase(j - 1)
                z_phase(j)
            agg_phase(BPC - 1)

    nc.compile()
    return nc


def prepare(x, W, att_src, att_dst, bias, edge_index):
    """Host-side sharding/slotting. Returns (program args, per-core in_maps,
    (N, D, node_map)) where node_map[c][row] = destination node of that
    output row (or -1 for padding)."""
    x = np.asarray(x, dtype=np.float32)
    W = np.asarray(W, dtype=np.float32)
    att_src = np.asarray(att_src, dtype=np.float32)
    att_dst = np.asarray(att_dst, dtype=np.float32)
    bias = np.asarray(bias, dtype=np.float32)
    ei = np.asarray(edge_index)

    N, D = x.shape
    assert D == P

    # self-loops FIRST so they land at chunk 0 of every destination run
    loop = np.arange(N, dtype=np.int64)
    src = np.concatenate([loop, ei[0]]).astype(np.int32)
    dst = np.concatenate([loop, ei[1]]).astype(np.int32)
    order = np.argsort(dst, kind="stable")
    src_s, dst_s = src[order], dst[order]

    deg = np.bincount(dst_s, minlength=N)
    runstart = np.zeros(N, dtype=np.int64)
    runstart[1:] = np.cumsum(deg)[:-1]

    NB = (N + P - 1) // P
    BPC = (NB + NCORES - 1) // NCORES
    NBP = BPC * NCORES

    # order destinations by degree (desc) -> blocks of near-equal degree
    perm = np.argsort(-deg, kind="stable").astype(np.int32)
    SENT = N
    blk_nodes = np.full((NBP, P), SENT, dtype=np.int32)
    blk_nodes.reshape(-1)[:N] = perm
    blk_deg = np.where(blk_nodes == SENT, 0,
                       deg[np.minimum(blk_nodes, N - 1)])
    blk_K = blk_deg.max(axis=1)

    # snake-deal blocks (sorted by K desc == index order) to (pos, core)
    KS = []
    core_blocks = [[] for _ in range(NCORES)]
    for p in range(BPC):
        idxs = list(range(p * NCORES, (p + 1) * NCORES))
        if p % 2 == 1:
            idxs = idxs[::-1]
        for c in range(NCORES):
            core_blocks[c].append(idxs[c])
        KS.append(int(max(max(blk_K[i] for i in idxs), 1)))
    CT = int(sum(KS))
    offs = np.concatenate([[0], np.cumsum(KS)]).astype(int)

    x16 = np.vstack([x.astype(np.float16), np.zeros((1, D), np.float16)])

    WT = np.ascontiguousarray(W.T)
    asc = np.ascontiguousarray(np.stack([att_src, att_dst], axis=1))
    brpc = np.ascontiguousarray(np.broadcast_to(bias, (P, P)))

    in_maps = []
    node_map = []
    for c in range(NCORES):
        grid = np.full((CT, P), SENT, dtype=np.int32)
        rows = np.empty(BPC * P, dtype=np.int32)
        for p in range(BPC):
            b = core_blocks[c][p]
            nodes = blk_nodes[b]
            rows[p * P:(p + 1) * P] = np.where(nodes == SENT, -1, nodes)
            real = np.nonzero(nodes != SENT)[0]
            if len(real) == 0:
                continue
            nd = nodes[real]
            dg_ = deg[nd]
            # edge t of node nd -> chunk offs[p]+t, partition real slot
            tot = dg_.sum()
            qs = np.repeat(real, dg_)
            ks = (np.arange(tot) -
                  np.repeat(np.cumsum(dg_) - dg_, dg_)).astype(np.int64)
            es = (np.repeat(runstart[nd], dg_) + ks)
            grid[offs[p] + ks, qs] = src_s[es]
        xg = x16[grid]  # [CT, P(slot), D]
        xsc = np.ascontiguousarray(xg.transpose(1, 0, 2)).reshape(P, CT * P)
        xTc = np.ascontiguousarray(xg.transpose(2, 0, 1)).reshape(P, CT * P)
        mc = np.where(grid == SENT, np.float32(-1e30), np.float32(0.0))
        mc = np.ascontiguousarray(mc.T)
        in_maps.append({"xs": xsc, "xsT": xTc, "msk": mc, "W": W, "WT": WT,
                        "asc": asc, "brp": brpc})
        node_map.append(rows)
    return (BPC, KS, CT), in_maps, (N, D, node_map)


def kernel(x, W, att_src, att_dst, bias, edge_index):
    from concourse.bass_utils import run_bass_kernel_spmd

    args, in_maps, (N, D, node_map) = prepare(x, W, att_src, att_dst, bias,
                                              edge_index)
    nc = build_program(*args)
    res = run_bass_kernel_spmd(nc, in_maps, list(range(NCORES)))

    out = np.empty((N, D), dtype=np.float32)
    for c in range(NCORES):
        rows = node_map[c]
        valid = rows >= 0
        out[rows[valid]] = res.results[c]["out"][valid]
    return out


# revision 11
# speedup vs baseline: 13.2808x; 1.0176x over previous
"""GATConv (PyG defaults, heads=1) Trainium2 Bass kernel.

Strategy (8 NeuronCores, edge-parallel over dst-sorted slots):
  Identity: out[dst] = (sum_e alpha_e * x[src_e]) @ W + bias -- the W GEMM
  distributes over the softmax-weighted aggregation, so we aggregate
  weighted *x* rows per destination first and multiply by W once per
  128-destination block.

  Host: prepend self-loops, sort edges by destination. Destinations are
  ordered by degree (descending) and packed into 128-node blocks so each
  block's chunk count K_b ~= its mean degree (kills max-degree padding).
  Blocks are dealt snake-wise to the 8 cores; the per-position chunk
  count KS[p] (max over cores) is compile-time so one SPMD program fits
  all cores. Each edge occupies (chunk k, partition q=dst slot); chunk 0
  is the self-loop. Host packs TWO fp16 layouts of x[src] per chunk:
    xs  [slot=128, feat=128]  (lhsT of the aggregation matmul)
    xsT [feat=128, slot=128]  (lhsT of the per-slot logit matmul)
  Padded slots carry x=0 plus an additive -1e30 logit mask.

  Device, per core (BPC blocks, K_j chunks each):
    - a_s per slot via PE: matmul(lhsT=xsT_k, rhs=W@att_src) -> [slot,1]
      PSUM column (W@att_src / W@att_dst computed on device once).
    - per block: z = a_s + a_d + mask; leaky-relu; per-dst max m (exact
      segment softmax, like the reference); ex = exp(lz - m) in [0,1].
    - dg_k = diag(ex_k) fp16, built on Vector/Scalar engines (tunable
      split); PE accumulates xaccT[c,d] += xs_k[slot,c]*dg_k[slot,d].
    - finalize: po = xaccT.T @ W (fp32); out = po/sum(ex) + bias.
  Outputs are un-permuted on the host (degree-sort is a permutation).
"""

import os
import sys

import numpy as np

sys.path.insert(0, "/opt/trn_rl_repo")

P = 128
NEG_SLOPE = 0.2
NCORES = 8

# dg-build engine split: V=Vector(DVE), A=Scalar(ACT), cycled per chunk
DG_PATTERN = "VVAVVAVA"


def build_program(BPC, KS, CT):
    from concourse import bacc, bass, mybir, tile

    f32 = mybir.dt.float32
    f16 = mybir.dt.float16
    Alu = mybir.AluOpType
    Act = mybir.ActivationFunctionType
    Ax = mybir.AxisListType
    KMAX = max(KS)
    offs = np.concatenate([[0], np.cumsum(KS)]).astype(int)

    nc = bacc.Bacc(None)

    xs = nc.declare_dram_parameter("xs", [P, CT * P], f16, isOutput=False)
    xsT = nc.declare_dram_parameter("xsT", [P, CT * P], f16, isOutput=False)
    msk = nc.declare_dram_parameter("msk", [P, CT], f32, isOutput=False)
    Wp = nc.declare_dram_parameter("W", [P, P], f32, isOutput=False)
    WTp = nc.declare_dram_parameter("WT", [P, P], f32, isOutput=False)
    ascp = nc.declare_dram_parameter("asc", [P, 2], f32, isOutput=False)
    brp = nc.declare_dram_parameter("brp", [P, P], f32, isOutput=False)
    outp = nc.declare_dram_parameter("out", [BPC * P, P], f32, isOutput=True)

    with tile.TileContext(nc) as tc:
        with (
            tc.tile_pool(name="const", bufs=1) as cpool,
            tc.tile_pool(name="pset", bufs=1, space="PSUM") as pset,
            tc.tile_pool(name="xseg", bufs=2) as xpool,
            tc.tile_pool(name="xTseg", bufs=2) as xTpool,
            tc.tile_pool(name="psas", bufs=2, space="PSUM") as psas,
            tc.tile_pool(name="blk", bufs=2) as bpool,
            tc.tile_pool(name="dg", bufs=12) as dpool,
            tc.tile_pool(name="psacc", bufs=2, space="PSUM") as psa,
            tc.tile_pool(name="pso", bufs=2, space="PSUM") as pso,
            tc.tile_pool(name="fin", bufs=2) as fpool,
        ):
            # ---- constants ----
            W_sb = cpool.tile([P, P], f32)
            nc.sync.dma_start(out=W_sb[:], in_=Wp[:])
            WT_sb = cpool.tile([P, P], f32)
            nc.sync.dma_start(out=WT_sb[:], in_=WTp[:])
            asc_sb = cpool.tile([P, 2], f32)
            nc.sync.dma_start(out=asc_sb[:], in_=ascp[:])
            brp_sb = cpool.tile([P, P], f32)
            nc.sync.dma_start(out=brp_sb[:], in_=brp[:])
            msk_sb = cpool.tile([P, CT], f32)
            nc.sync.dma_start(out=msk_sb[:], in_=msk[:])

            iota_row = cpool.tile([P, P], f16)
            nc.gpsimd.iota(iota_row[:], pattern=[[1, P]], base=0,
                           channel_multiplier=0,
                           allow_small_or_imprecise_dtypes=True)
            iota_col = cpool.tile([P, 1], f32)
            nc.gpsimd.iota(iota_col[:], pattern=[[1, 1]], base=0,
                           channel_multiplier=1,
                           allow_small_or_imprecise_dtypes=True)
            I01 = cpool.tile([P, P], f16)
            nc.vector.tensor_scalar(I01[:], iota_row[:], iota_col[:, 0:1],
                                    None, Alu.is_equal)

            # wsd[c, 0:2] = [W @ att_src, W @ att_dst][c]
            wsd_ps = pset.tile([P, 2], f32, tag="wsd")
            nc.tensor.matmul(out=wsd_ps[:], lhsT=WT_sb[:], rhs=asc_sb[:],
                             start=True, stop=True)
            wsd16 = cpool.tile([P, 2], f16)
            nc.scalar.activation(out=wsd16[:], in_=wsd_ps[:], func=Act.Copy)

            # ---- software pipeline over blocks: per iteration j emit
            #   A_j  (DMA + per-slot logit matmuls on PE)
            #   G_j-1 (dg builds + aggregation matmuls + finalize)
            #   Z_j  (z -> lrelu -> max -> exp -> denom batch)
            # so the DVE/ACT dg work of block j-1 is queued BEFORE the Z_j
            # ops that stall on A_j's matmuls (avoids head-of-line blocking).
            state = [None] * BPC

            def as_phase(j):
                K = KS[j]
                o = offs[j]
                xseg = xpool.tile([P, KMAX * P], f16, tag="xseg")
                nc.sync.dma_start(out=xseg[:, 0:K * P],
                                  in_=xs[:, o * P:(o + K) * P])
                xTseg = xTpool.tile([P, KMAX * P], f16, tag="xTseg")
                nc.sync.dma_start(out=xTseg[:, 0:K * P],
                                  in_=xsT[:, o * P:(o + K) * P])
                as_ps = psas.tile([P, KMAX + 1], f32, tag="asps")
                for k in range(K):
                    nc.tensor.matmul(out=as_ps[:, k:k + 1],
                                     lhsT=xTseg[:, k * P:(k + 1) * P],
                                     rhs=wsd16[:, 0:1], start=True, stop=True,
                                     skip_group_check=True)
                ad_ps = as_ps[:, KMAX:KMAX + 1]
                nc.tensor.matmul(out=ad_ps, lhsT=xTseg[:, 0:P],
                                 rhs=wsd16[:, 1:2], start=True, stop=True,
                                 skip_group_check=True)
                state[j] = [xseg, as_ps, None, None]

            def z_phase(j):
                K = KS[j]
                o = offs[j]
                as_ps = state[j][1]
                ad_ps = as_ps[:, KMAX:KMAX + 1]
                # z = a_s + a_d + mask; lz = lrelu(z); ex = exp(lz - max)
                ad_sb = bpool.tile([P, 1], f32, tag="adsb")
                nc.vector.tensor_scalar(ad_sb[:], ad_ps, 0.0, None,
                                        Alu.add)
                z_blk = bpool.tile([P, KMAX], f32, tag="z")
                nc.vector.tensor_scalar(z_blk[:, 0:K], as_ps[:, 0:K],
                                        ad_sb[:, 0:1], None, Alu.add)
                z2_blk = bpool.tile([P, KMAX], f32, tag="z2")
                nc.vector.tensor_tensor(out=z2_blk[:, 0:K], in0=z_blk[:, 0:K],
                                        in1=msk_sb[:, o:o + K], op=Alu.add)
                lz_blk = bpool.tile([P, KMAX], f32, tag="lz")
                nc.vector.scalar_tensor_tensor(
                    out=lz_blk[:, 0:K], in0=z2_blk[:, 0:K], scalar=NEG_SLOPE,
                    in1=z2_blk[:, 0:K], op0=Alu.mult, op1=Alu.max)
                m_col = bpool.tile([P, 1], f32, tag="m")
                nc.vector.tensor_reduce(out=m_col[:], in_=lz_blk[:, 0:K],
                                        axis=Ax.X, op=Alu.max)
                lzm_blk = bpool.tile([P, KMAX], f32, tag="lzm")
                nc.vector.tensor_scalar(lzm_blk[:, 0:K], lz_blk[:, 0:K],
                                        m_col[:, 0:1], None, Alu.subtract)
                ex_blk = bpool.tile([P, KMAX], f32, tag="ex")
                nc.scalar.activation(out=ex_blk[:, 0:K], in_=lzm_blk[:, 0:K],
                                     func=Act.Exp)
                dn_col = bpool.tile([P, 1], f32, tag="dn")
                nc.vector.tensor_reduce(out=dn_col[:], in_=ex_blk[:, 0:K],
                                        axis=Ax.X, op=Alu.add)
                rc_col = bpool.tile([P, 1], f32, tag="rc")
                nc.vector.reciprocal(out=rc_col[:], in_=dn_col[:])
                state[j][2] = ex_blk
                state[j][3] = rc_col

            def agg_phase(j):
                K = KS[j]
                xseg, _, ex_blk, rc_col = state[j]
                state[j] = None
                xacc_ps = psa.tile([P, P], f32, tag="xacc")
                for k in range(K):
                    dg = dpool.tile([P, P], f16, tag="dg")
                    if DG_PATTERN[k % len(DG_PATTERN)] == "A":
                        nc.scalar.activation(out=dg[:], in_=I01[:],
                                             func=Act.Copy,
                                             scale=ex_blk[:, k:k + 1])
                    else:
                        nc.vector.tensor_scalar(
                            dg[:], I01[:], ex_blk[:, k:k + 1], None, Alu.mult)
                    nc.tensor.matmul(out=xacc_ps[:],
                                     lhsT=xseg[:, k * P:(k + 1) * P],
                                     rhs=dg[:], start=(k == 0),
                                     stop=(k == K - 1),
                                     skip_group_check=True)

                xacc_sb = fpool.tile([P, P], f32, tag="xaccsb")
                nc.scalar.activation(out=xacc_sb[:], in_=xacc_ps[:],
                                     func=Act.Copy)
                po_ps = pso.tile([P, P], f32, tag="po")
                nc.tensor.matmul(out=po_ps[:], lhsT=xacc_sb[:], rhs=W_sb[:],
                                 start=True, stop=True, skip_group_check=True)
                ob = fpool.tile([P, P], f32, tag="ob")
                nc.vector.scalar_tensor_tensor(
                    out=ob[:], in0=po_ps[:], scalar=rc_col[:, 0:1],
                    in1=brp_sb[:], op0=Alu.mult, op1=Alu.add)
                nc.sync.dma_start(out=outp[j * P:(j + 1) * P, :], in_=ob[:])

            for j in range(BPC):
                as_phase(j)
                if j > 0:
                    agg_phase(j - 1)
                z_phase(j)
            agg_phase(BPC - 1)

    nc.compile()
    return nc


def prepare(x, W, att_src, att_dst, bias, edge_index):
    """Host-side sharding/slotting. Returns (program args, per-core in_maps,
    (N, D, node_map)) where node_map[c][row] = destination node of that
    output row (or -1 for padding)."""
    x = np.asarray(x, dtype=np.float32)
    W = np.asarray(W, dtype=np.float32)
    att_src = np.asarray(att_src, dtype=np.float32)
    att_dst = np.asarray(att_dst, dtype=np.float32)
    bias = np.asarray(bias, dtype=np.float32)
    ei = np.asarray(edge_index)

    N, D = x.shape
    assert D == P

    # self-loops FIRST so they land at chunk 0 of every destination run
    loop = np.arange(N, dtype=np.int64)
    src = np.concatenate([loop, ei[0]]).astype(np.int32)
    dst = np.concatenate([loop, ei[1]]).astype(np.int32)
    order = np.argsort(dst, kind="stable")
    src_s, dst_s = src[order], dst[order]

    deg = np.bincount(dst_s, minlength=N)
    runstart = np.zeros(N, dtype=np.int64)
    runstart[1:] = np.cumsum(deg)[:-1]

    NB = (N + P - 1) // P
    BPC = (NB + NCORES - 1) // NCORES
    NBP = BPC * NCORES

    # order destinations by degree (desc) -> blocks of near-equal degree
    perm = np.argsort(-deg, kind="stable").astype(np.int32)
    SENT = N
    blk_nodes = np.full((NBP, P), SENT, dtype=np.int32)
    blk_nodes.reshape(-1)[:N] = perm
    blk_deg = np.where(blk_nodes == SENT, 0,
                       deg[np.minimum(blk_nodes, N - 1)])
    blk_K = blk_deg.max(axis=1)

    # snake-deal blocks (sorted by K desc == index order) to (pos, core)
    KS = []
    core_blocks = [[] for _ in range(NCORES)]
    for p in range(BPC):
        idxs = list(range(p * NCORES, (p + 1) * NCORES))
        if p % 2 == 1:
            idxs = idxs[::-1]
        for c in range(NCORES):
            core_blocks[c].append(idxs[c])
        KS.append(int(max(max(blk_K[i] for i in idxs), 1)))
    CT = int(sum(KS))
    offs = np.concatenate([[0], np.cumsum(KS)]).astype(int)

    x16 = np.vstack([x.astype(np.float16), np.zeros((1, D), np.float16)])

    WT = np.ascontiguousarray(W.T)
    asc = np.ascontiguousarray(np.stack([att_src, att_dst], axis=1))
    brpc = np.ascontiguousarray(np.broadcast_to(bias, (P, P)))

    in_maps = []
    node_map = []
    for c in range(NCORES):
        grid = np.full((CT, P), SENT, dtype=np.int32)
        rows = np.empty(BPC * P, dtype=np.int32)
        for p in range(BPC):
            b = core_blocks[c][p]
            nodes = blk_nodes[b]
            rows[p * P:(p + 1) * P] = np.where(nodes == SENT, -1, nodes)
            real = np.nonzero(nodes != SENT)[0]
            if len(real) == 0:
                continue
            nd = nodes[real]
            dg_ = deg[nd]
            # edge t of node nd -> chunk offs[p]+t, partition real slot
            tot = dg_.sum()
            qs = np.repeat(real, dg_)
            ks = (np.arange(tot) -
                  np.repeat(np.cumsum(dg_) - dg_, dg_)).astype(np.int64)
            es = (np.repeat(runstart[nd], dg_) + ks)
            grid[offs[p] + ks, qs] = src_s[es]
        xg = x16[grid]  # [CT, P(slot), D]
        xsc = np.ascontiguousarray(xg.transpose(1, 0, 2)).reshape(P, CT * P)
        xTc = np.ascontiguousarray(xg.transpose(2, 0, 1)).reshape(P, CT * P)
        mc = np.where(grid == SENT, np.float32(-1e30), np.float32(0.0))
        mc = np.ascontiguousarray(mc.T)
        in_maps.append({"xs": xsc, "xsT": xTc, "msk": mc, "W": W, "WT": WT,
                        "asc": asc, "brp": brpc})
        node_map.append(rows)
    return (BPC, KS, CT), in_maps, (N, D, node_map)


def kernel(x, W, att_src, att_dst, bias, edge_index):
    from concourse.bass_utils import run_bass_kernel_spmd

    args, in_maps, (N, D, node_map) = prepare(x, W, att_src, att_dst, bias,
                                              edge_index)
    nc = build_program(*args)
    res = run_bass_kernel_spmd(nc, in_maps, list(range(NCORES)))

    out = np.empty((N, D), dtype=np.float32)
    for c in range(NCORES):
        rows = node_map[c]
        valid = rows >= 0
        out[rows[valid]] = res.results[c]["out"][valid]
    return out


# revision 14
# speedup vs baseline: 13.6627x; 1.0288x over previous
"""GATConv (PyG defaults, heads=1) Trainium2 Bass kernel.

Strategy (8 NeuronCores, edge-parallel over dst-sorted slots):
  Identity: out[dst] = (sum_e alpha_e * x[src_e]) @ W + bias -- the W GEMM
  distributes over the softmax-weighted aggregation, so we aggregate
  weighted *x* rows per destination first and multiply by W once per
  128-destination block.

  Host: prepend self-loops, sort edges by destination. Destinations are
  ordered by degree (descending) and packed into 128-node blocks so each
  block's chunk count K_b ~= its mean degree (kills max-degree padding).
  Blocks are dealt snake-wise to the 8 cores; the per-position chunk
  count KS[p] (max over cores) is compile-time so one SPMD program fits
  all cores. Each edge occupies (chunk k, partition q=dst slot); chunk 0
  is the self-loop. Host packs TWO fp16 layouts of x[src] per chunk:
    xs  [slot=128, feat=128]  (lhsT of the aggregation matmul)
    xsT [feat=128, slot=128]  (lhsT of the per-slot logit matmul)
  Padded slots carry x=0 plus an additive -1e30 logit mask.

  Device, per core (BPC blocks, K_j chunks each):
    - a_s per slot via PE: matmul(lhsT=xsT_k, rhs=W@att_src) -> [slot,1]
      PSUM column (W@att_src / W@att_dst computed on device once).
    - per block: z = a_s + a_d + mask; leaky-relu; per-dst max m (exact
      segment softmax, like the reference); ex = exp(lz - m) in [0,1].
    - dg_k = diag(ex_k) fp16, built on Vector/Scalar engines (tunable
      split); PE accumulates xaccT[c,d] += xs_k[slot,c]*dg_k[slot,d].
    - finalize: po = xaccT.T @ W (fp32); out = po/sum(ex) + bias.
  Outputs are un-permuted on the host (degree-sort is a permutation).
"""

import os
import sys

import numpy as np

sys.path.insert(0, "/opt/trn_rl_repo")

P = 128
NEG_SLOPE = 0.2
NCORES = 8

# dg-build engine split: V=Vector(DVE), A=Scalar(ACT), cycled per chunk
DG_PATTERN = "VVAVVAVA"


def build_program(BPC, KS, CT):
    from concourse import bacc, bass, mybir, tile

    f32 = mybir.dt.float32
    f16 = mybir.dt.float16
    Alu = mybir.AluOpType
    Act = mybir.ActivationFunctionType
    Ax = mybir.AxisListType
    KMAX = max(KS)
    offs = np.concatenate([[0], np.cumsum(KS)]).astype(int)

    nc = bacc.Bacc(None)

    xs = nc.declare_dram_parameter("xs", [P, CT * P], f16, isOutput=False)
    xsT = nc.declare_dram_parameter("xsT", [P, CT * P], f16, isOutput=False)
    msk = nc.declare_dram_parameter("msk", [P, CT], f32, isOutput=False)
    Wp = nc.declare_dram_parameter("W", [P, P], f32, isOutput=False)
    WTp = nc.declare_dram_parameter("WT", [P, P], f32, isOutput=False)
    ascp = nc.declare_dram_parameter("asc", [P, 2], f32, isOutput=False)
    brp = nc.declare_dram_parameter("brp", [P, P], f32, isOutput=False)
    outp = nc.declare_dram_parameter("out", [BPC * P, P], f32, isOutput=True)

    with tile.TileContext(nc) as tc:
        with (
            tc.tile_pool(name="const", bufs=1) as cpool,
            tc.tile_pool(name="pset", bufs=1, space="PSUM") as pset,
            tc.tile_pool(name="xseg", bufs=3) as xpool,
            tc.tile_pool(name="xTseg", bufs=2) as xTpool,
            tc.tile_pool(name="psas", bufs=2, space="PSUM") as psas,
            tc.tile_pool(name="blk", bufs=3) as bpool,
            tc.tile_pool(name="dg", bufs=12) as dpool,
            tc.tile_pool(name="psacc", bufs=2, space="PSUM") as psa,
            tc.tile_pool(name="pso", bufs=2, space="PSUM") as pso,
            tc.tile_pool(name="fin", bufs=2) as fpool,
        ):
            # ---- constants ----
            W_sb = cpool.tile([P, P], f32)
            nc.sync.dma_start(out=W_sb[:], in_=Wp[:])
            WT_sb = cpool.tile([P, P], f32)
            nc.sync.dma_start(out=WT_sb[:], in_=WTp[:])
            asc_sb = cpool.tile([P, 2], f32)
            nc.sync.dma_start(out=asc_sb[:], in_=ascp[:])
            brp_sb = cpool.tile([P, P], f32)
            nc.sync.dma_start(out=brp_sb[:], in_=brp[:])
            msk_sb = cpool.tile([P, CT], f32)
            nc.sync.dma_start(out=msk_sb[:], in_=msk[:])

            iota_row = cpool.tile([P, P], f16)
            nc.gpsimd.iota(iota_row[:], pattern=[[1, P]], base=0,
                           channel_multiplier=0,
                           allow_small_or_imprecise_dtypes=True)
            iota_col = cpool.tile([P, 1], f32)
            nc.gpsimd.iota(iota_col[:], pattern=[[1, 1]], base=0,
                           channel_multiplier=1,
                           allow_small_or_imprecise_dtypes=True)
            I01 = cpool.tile([P, P], f16)
            nc.vector.tensor_scalar(I01[:], iota_row[:], iota_col[:, 0:1],
                                    None, Alu.is_equal)

            # wsd[c, 0:2] = [W @ att_src, W @ att_dst][c]
            wsd_ps = pset.tile([P, 2], f32, tag="wsd")
            nc.tensor.matmul(out=wsd_ps[:], lhsT=WT_sb[:], rhs=asc_sb[:],
                             start=True, stop=True)
            wsd16 = cpool.tile([P, 2], f16)
            nc.scalar.activation(out=wsd16[:], in_=wsd_ps[:], func=Act.Copy)

            # ---- software pipeline over blocks: per iteration j emit
            #   A_j  (DMA + per-slot logit matmuls on PE)
            #   G_j-1 (dg builds + aggregation matmuls + finalize)
            #   Z_j  (z -> lrelu -> max -> exp -> denom batch)
            # so the DVE/ACT dg work of block j-1 is queued BEFORE the Z_j
            # ops that stall on A_j's matmuls (avoids head-of-line blocking).
            state = [None] * BPC

            def as_phase(j):
                K = KS[j]
                o = offs[j]
                xseg = xpool.tile([P, KMAX * P], f16, tag="xseg")
                nc.sync.dma_start(out=xseg[:, 0:K * P],
                                  in_=xs[:, o * P:(o + K) * P])
                xTseg = xTpool.tile([P, KMAX * P], f16, tag="xTseg")
                nc.sync.dma_start(out=xTseg[:, 0:K * P],
                                  in_=xsT[:, o * P:(o + K) * P])
                as_ps = psas.tile([P, KMAX + 1], f32, tag="asps")
                for k in range(K):
                    nc.tensor.matmul(out=as_ps[:, k:k + 1],
                                     lhsT=xTseg[:, k * P:(k + 1) * P],
                                     rhs=wsd16[:, 0:1], start=True, stop=True,
                                     skip_group_check=True)
                ad_ps = as_ps[:, KMAX:KMAX + 1]
                nc.tensor.matmul(out=ad_ps, lhsT=xTseg[:, 0:P],
                                 rhs=wsd16[:, 1:2], start=True, stop=True,
                                 skip_group_check=True)
                state[j] = [xseg, as_ps, None, None]

            def z_phase(j):
                K = KS[j]
                o = offs[j]
                as_ps = state[j][1]
                ad_ps = as_ps[:, KMAX:KMAX + 1]
                # z = a_s + a_d + mask; lz = lrelu(z); ex = exp(lz - max)
                ad_sb = bpool.tile([P, 1], f32, tag="adsb")
                nc.vector.tensor_scalar(ad_sb[:], ad_ps, 0.0, None,
                                        Alu.add)
                z_blk = bpool.tile([P, KMAX], f32, tag="z")
                nc.vector.tensor_scalar(z_blk[:, 0:K], as_ps[:, 0:K],
                                        ad_sb[:, 0:1], None, Alu.add)
                z2_blk = bpool.tile([P, KMAX], f32, tag="z2")
                nc.vector.tensor_tensor(out=z2_blk[:, 0:K], in0=z_blk[:, 0:K],
                                        in1=msk_sb[:, o:o + K], op=Alu.add)
                lz_blk = bpool.tile([P, KMAX], f32, tag="lz")
                nc.vector.scalar_tensor_tensor(
                    out=lz_blk[:, 0:K], in0=z2_blk[:, 0:K], scalar=NEG_SLOPE,
                    in1=z2_blk[:, 0:K], op0=Alu.mult, op1=Alu.max)
                m_col = bpool.tile([P, 1], f32, tag="m")
                nc.vector.tensor_reduce(out=m_col[:], in_=lz_blk[:, 0:K],
                                        axis=Ax.X, op=Alu.max)
                lzm_blk = bpool.tile([P, KMAX], f32, tag="lzm")
                nc.vector.tensor_scalar(lzm_blk[:, 0:K], lz_blk[:, 0:K],
                                        m_col[:, 0:1], None, Alu.subtract)
                ex_blk = bpool.tile([P, KMAX], f32, tag="ex")
                nc.scalar.activation(out=ex_blk[:, 0:K], in_=lzm_blk[:, 0:K],
                                     func=Act.Exp)
                dn_col = bpool.tile([P, 1], f32, tag="dn")
                nc.vector.tensor_reduce(out=dn_col[:], in_=ex_blk[:, 0:K],
                                        axis=Ax.X, op=Alu.add)
                rc_col = bpool.tile([P, 1], f32, tag="rc")
                nc.vector.reciprocal(out=rc_col[:], in_=dn_col[:])
                state[j][2] = ex_blk
                state[j][3] = rc_col

            def agg_phase(j):
                K = KS[j]
                xseg, _, ex_blk, rc_col = state[j]
                state[j] = None
                xacc_ps = psa.tile([P, P], f32, tag="xacc")
                for k in range(K):
                    dg = dpool.tile([P, P], f16, tag="dg")
                    if DG_PATTERN[k % len(DG_PATTERN)] == "A":
                        nc.scalar.activation(out=dg[:], in_=I01[:],
                                             func=Act.Copy,
                                             scale=ex_blk[:, k:k + 1])
                    else:
                        nc.vector.tensor_scalar(
                            dg[:], I01[:], ex_blk[:, k:k + 1], None, Alu.mult)
                    nc.tensor.matmul(out=xacc_ps[:],
                                     lhsT=xseg[:, k * P:(k + 1) * P],
                                     rhs=dg[:], start=(k == 0),
                                     stop=(k == K - 1),
                                     skip_group_check=True)

                xacc_sb = fpool.tile([P, P], f32, tag="xaccsb")
                nc.scalar.activation(out=xacc_sb[:], in_=xacc_ps[:],
                                     func=Act.Copy)
                po_ps = pso.tile([P, P], f32, tag="po")
                nc.tensor.matmul(out=po_ps[:], lhsT=xacc_sb[:], rhs=W_sb[:],
                                 start=True, stop=True, skip_group_check=True)
                ob = fpool.tile([P, P], f32, tag="ob")
                nc.vector.scalar_tensor_tensor(
                    out=ob[:], in0=po_ps[:], scalar=rc_col[:, 0:1],
                    in1=brp_sb[:], op0=Alu.mult, op1=Alu.add)
                nc.sync.dma_start(out=outp[j * P:(j + 1) * P, :], in_=ob[:])

            for j in range(BPC):
                as_phase(j)
                if j > 1:
                    agg_phase(j - 2)
                z_phase(j)
            agg_phase(BPC - 2)
            agg_phase(BPC - 1)

    nc.compile()
    return nc


def prepare(x, W, att_src, att_dst, bias, edge_index):
    """Host-side sharding/slotting. Returns (program args, per-core in_maps,
    (N, D, node_map)) where node_map[c][row] = destination node of that
    output row (or -1 for padding)."""
    x = np.asarray(x, dtype=np.float32)
    W = np.asarray(W, dtype=np.float32)
    att_src = np.asarray(att_src, dtype=np.float32)
    att_dst = np.asarray(att_dst, dtype=np.float32)
    bias = np.asarray(bias, dtype=np.float32)
    ei = np.asarray(edge_index)

    N, D = x.shape
    assert D == P

    # self-loops FIRST so they land at chunk 0 of every destination run
    loop = np.arange(N, dtype=np.int64)
    src = np.concatenate([loop, ei[0]]).astype(np.int32)
    dst = np.concatenate([loop, ei[1]]).astype(np.int32)
    order = np.argsort(dst, kind="stable")
    src_s, dst_s = src[order], dst[order]

    deg = np.bincount(dst_s, minlength=N)
    runstart = np.zeros(N, dtype=np.int64)
    runstart[1:] = np.cumsum(deg)[:-1]

    NB = (N + P - 1) // P
    BPC = (NB + NCORES - 1) // NCORES
    NBP = BPC * NCORES

    # order destinations by degree (desc) -> blocks of near-equal degree
    perm = np.argsort(-deg, kind="stable").astype(np.int32)
    SENT = N
    blk_nodes = np.full((NBP, P), SENT, dtype=np.int32)
    blk_nodes.reshape(-1)[:N] = perm
    blk_deg = np.where(blk_nodes == SENT, 0,
                       deg[np.minimum(blk_nodes, N - 1)])
    blk_K = blk_deg.max(axis=1)

    # snake-deal blocks (sorted by K desc == index order) to (pos, core)
    KS = []
    core_blocks = [[] for _ in range(NCORES)]
    for p in range(BPC):
        idxs = list(range(p * NCORES, (p + 1) * NCORES))
        if p % 2 == 1:
            idxs = idxs[::-1]
        for c in range(NCORES):
            core_blocks[c].append(idxs[c])
        KS.append(int(max(max(blk_K[i] for i in idxs), 1)))
    CT = int(sum(KS))
    offs = np.concatenate([[0], np.cumsum(KS)]).astype(int)

    x16 = np.vstack([x.astype(np.float16), np.zeros((1, D), np.float16)])

    WT = np.ascontiguousarray(W.T)
    asc = np.ascontiguousarray(np.stack([att_src, att_dst], axis=1))
    brpc = np.ascontiguousarray(np.broadcast_to(bias, (P, P)))

    in_maps = []
    node_map = []
    for c in range(NCORES):
        grid = np.full((CT, P), SENT, dtype=np.int32)
        rows = np.empty(BPC * P, dtype=np.int32)
        for p in range(BPC):
            b = core_blocks[c][p]
            nodes = blk_nodes[b]
            rows[p * P:(p + 1) * P] = np.where(nodes == SENT, -1, nodes)
            real = np.nonzero(nodes != SENT)[0]
            if len(real) == 0:
                continue
            nd = nodes[real]
            dg_ = deg[nd]
            # edge t of node nd -> chunk offs[p]+t, partition real slot
            tot = dg_.sum()
            qs = np.repeat(real, dg_)
            ks = (np.arange(tot) -
                  np.repeat(np.cumsum(dg_) - dg_, dg_)).astype(np.int64)
            es = (np.repeat(runstart[nd], dg_) + ks)
            grid[offs[p] + ks, qs] = src_s[es]
        xg = x16[grid]  # [CT, P(slot), D]
        xsc = np.ascontiguousarray(xg.transpose(1, 0, 2)).reshape(P, CT * P)
        xTc = np.ascontiguousarray(xg.transpose(2, 0, 1)).reshape(P, CT * P)
        mc = np.where(grid == SENT, np.float32(-1e30), np.float32(0.0))
        mc = np.ascontiguousarray(mc.T)
        in_maps.append({"xs": xsc, "xsT": xTc, "msk": mc, "W": W, "WT": WT,
                        "asc": asc, "brp": brpc})
        node_map.append(rows)
    return (BPC, KS, CT), in_maps, (N, D, node_map)


def kernel(x, W, att_src, att_dst, bias, edge_index):
    from concourse.bass_utils import run_bass_kernel_spmd

    args, in_maps, (N, D, node_map) = prepare(x, W, att_src, att_dst, bias,
                                              edge_index)
    nc = build_program(*args)
    res = run_bass_kernel_spmd(nc, in_maps, list(range(NCORES)))

    out = np.empty((N, D), dtype=np.float32)
    for c in range(NCORES):
        rows = node_map[c]
        valid = rows >= 0
        out[rows[valid]] = res.results[c]["out"][valid]
    return out
